# revision 2
# baseline (speedup 1.0000x reference)
"""ContextualConv2d Trainium2 kernel.

Problem: grouped 3x3 conv (N=32, 128ci -> 256co, groups=4, 56x56, pad 1)
plus per-(batch,channel) context bias: out = conv(x, w) + (c @ cwT)[n,co]
+ bias[co].

Sharding (8 cores): core = (group-pair gp in {0,1}) x (batch quarter q in
{0..3}). Each core computes 8 images x 128 out-channels; the 128 co of
groups {2gp, 2gp+1} depend only on ci [64gp, 64gp+64).

Matmul scheme (v2): partitions 0..63 hold the image's 64 ci; partitions
64..127 hold the same data shifted down one image row (one on-chip
SBUF->SBUF DMA per image). A K=128 matmul then contracts TWO kernel rows
at once: lhsT rows 0..63 carry the (kh=0, kw) block-diagonal weight
block, rows 64..127 the (kh=1, kw) block. 3 such passes (kw = 0,1,2)
cover kh in {0,1}; 3 K=64 passes cover kh=2. 6 passes/tile instead of
the 9 of the one-position-per-pass layout: PE rows drop from 225792 to
150528 per core. Context/bias epilogue fused into the PSUM->SBUF copy,
alternating DVE/ACT engines.
"""

import numpy as np

from concourse import bass, mybir, tile
from concourse.vector_clock import ScopedClock
from concourse.bass_utils import run_bass_kernel_spmd

N, CIN, H, W = 32, 128, 56, 56
COUT, KH, KW = 256, 3, 3
GROUPS = 4
CDIM = 64
HP, WP = H + 2, W + 2
ROWS = 8              # output rows per n-tile
NT = H // ROWS        # 7 n-tiles per image
NFREE = ROWS * W      # 448 <= 512 fp32 PSUM bank limit
N_CORES = 8
IMGS = N // 4         # 8 images per core
CI = CIN // 2         # 64 input channels per core (2 groups)
CO = COUT // 2        # 128 output channels per core (2 groups)
NPASS = 6             # 3 dual-row (K=128) + 3 single-row (K=64) passes


class _TC(tile.TileContext):
    """This container's walrus accepts only one sem wait on a Drain
    (CTRL) instruction; TileContext's tail drain aggregates one wait per
    outstanding semaphore. Split them across sequential drains."""

    def _drain_and_barrier(self, tick_clock, wait_clock):
        drain_inst = self.nc.sync.drain()
        wait_clock.add_sem_waits(
            drain_inst.ins, ScopedClock({None: tick_clock.global_clock})
        )
        si = drain_inst.ins.sync_info
        if si is not None and len(si.on_wait) > 1:
            waits = list(si.on_wait)
            si.on_wait.clear()
            si.on_wait.append(waits[0])
            for w in waits[1:]:
                d2 = self.nc.sync.drain()
                d2.ins.sync_info = mybir.SyncInfo(on_wait=[w], on_update=[])
        self.nc.all_engine_barrier()
        assert self.sems is not None
        popped = self.nc._tile_sem_poison_stack.pop()
        assert popped is self._sem_poison
        self.nc.clear_and_free_semaphores(list(self.sems.allocated().values()))
        self.nc.all_engine_barrier()


_ws_ctr = [0]


def _split_waits(nc):
    """Walrus here caps sem waits at one per instruction; hoist extras
    onto injected same-engine NoOps placed just before the owner."""
    for fn in nc.m.functions:
        for blk in fn.blocks:
            insts = blk.instructions
            out = []
            changed = False
            for inst in insts:
                si = getattr(inst, "sync_info", None)
                if si is not None and si.on_wait and len(si.on_wait) > 1:
                    waits = list(si.on_wait)
                    for w in waits[:-1]:
                        _ws_ctr[0] += 1
                        out.append(
                            mybir.InstNoOp(
                                name=f"WSNOP-{_ws_ctr[0]}",
                                engine=inst.engine,
                                ins=[],
                                outs=[],
                                sync_info=mybir.SyncInfo(on_wait=[w], on_update=[]),
                                debug=inst.debug,
                            )
                        )
                        changed = True
                    si.on_wait.clear()
                    si.on_wait.append(waits[-1])
                out.append(inst)
            if changed:
                insts.clear()
                insts.extend(out)
    return nc


def build_program(loop_n: int = 0):
    """loop_n > 0 builds a benchmark variant: the conv body repeats
    loop_n times inside a hardware For_i so device time dominates the
    (RPC/transfer-heavy) wall clock. loop_n=0 is the production kernel."""
    f32 = mybir.dt.float32
    f32r = mybir.dt.float32r
    nc = bass.Bass("TRN2", target_bir_lowering=False, debug=False)
    xs = nc.declare_dram_parameter("xs", [IMGS, CI, HP, WP], f32r, isOutput=False)
    wb = nc.declare_dram_parameter("wb", [128, NPASS, CO], f32r, isOutput=False)
    cwb = nc.declare_dram_parameter("cwb", [CDIM + 1, CO], f32r, isOutput=False)
    cb = nc.declare_dram_parameter("cb", [CDIM + 1, IMGS], f32r, isOutput=False)
    y = nc.declare_dram_parameter("y", [IMGS, CO, H, W], f32, isOutput=True)

    with _TC(nc) as tc:
        with (
            tc.tile_pool(name="wp", bufs=1) as wpool,
            tc.tile_pool(name="xp", bufs=4) as xpool,
            tc.tile_pool(name="op", bufs=3) as opool,
            tc.tile_pool(name="psp", bufs=6, space="PSUM") as pspool,
            tc.tile_pool(name="psc", bufs=1, space="PSUM") as pscpool,
        ):
            wt = wpool.tile([128, NPASS, CO], f32r)
            nc.sync.dma_start(wt[:], wb[:])
            cwbt = wpool.tile([CDIM + 1, CO], f32r)
            nc.sync.dma_start(cwbt[:], cwb[:])
            cbt = wpool.tile([CDIM + 1, IMGS], f32r)
            nc.sync.dma_start(cbt[:], cb[:])

            # bctx[co, n] = sum_d c_weight[co,d] c[n,d] + bias[co]
            psc = pscpool.tile([CO, IMGS], f32)
            nc.tensor.matmul(psc[:, :], cwbt[:], cbt[:], start=True, stop=True)
            bctx = wpool.tile([CO, IMGS], f32)
            nc.vector.tensor_copy(bctx[:], psc[:, :])

            def conv_body():
                for i in range(IMGS):
                    xt = xpool.tile([128, HP, WP], f32r, name=f"xt{i}", tag="xt")
                    nc.sync.dma_start(xt[0:64], xs[i])
                    # partitions 64..127 = same image shifted down one row
                    nc.sync.dma_start(xt[64:128, 0 : HP - 1, :], xt[0:64, 1:HP, :])
                    ot = opool.tile([128, H * W], f32, name=f"ot{i}", tag="ot")
                    for t in range(NT):
                        ps = pspool.tile([128, NFREE], f32, name=f"ps{i}_{t}", tag="ps")
                        h0 = t * ROWS
                        for j in range(3):
                            # lower: (kh=0, kw=j); upper: (kh=1, kw=j)
                            nc.tensor.matmul(
                                ps[:, :],
                                wt[:, j, :],
                                xt[:, h0 : h0 + ROWS, j : j + W],
                                start=(j == 0),
                                stop=False,
                            )
                        for j in range(3):
                            # (kh=2, kw=j), lower 64 partitions only
                            nc.tensor.matmul(
                                ps[:, :],
                                wt[0:64, 3 + j, :],
                                xt[0:64, h0 + 2 : h0 + 2 + ROWS, j : j + W],
                                start=False,
                                stop=(j == 2),
                            )
                        o = ot[:, t * NFREE : (t + 1) * NFREE]
                        if t % 2 == 0:
                            nc.vector.tensor_scalar_add(o, ps[:, :], bctx[:, i : i + 1])
                        else:
                            nc.scalar.activation(
                                o, ps[:, :], mybir.ActivationFunctionType.Identity,
                                bias=bctx[:, i : i + 1],
                            )
                    nc.sync.dma_start(y[i].rearrange("c h w -> c (h w)"), ot[:])

            if loop_n > 0:
                with tc.For_i(0, loop_n, 1, hint_engines=(mybir.EngineType.PE,)):
                    conv_body()
            else:
                conv_body()
    _split_waits(nc)
    return nc


_prog_cache = {}


def _get_program():
    if "nc" not in _prog_cache:
        _prog_cache["nc"] = build_program()
    return _prog_cache["nc"]


def _shard_inputs(x, c, weight, bias, c_weight):
    """Build the per-core input dicts (pure layout prep, no math)."""
    xpad = np.zeros((N, CIN, HP, WP), np.float32)
    xpad[:, :, 1 : H + 1, 1 : W + 1] = x

    # Pass-major block-diagonal weights for each group pair.
    # Pass j in 0..2: rows 0..63 = (kh=0, kw=j) block, rows 64..127 =
    # (kh=1, kw=j) block. Pass 3+j: rows 0..63 = (kh=2, kw=j), rows
    # 64..127 unused (K=64 matmul). Block: ci_loc 32g..32g+31 ->
    # co_loc 64g..64g+63 for g in {0,1}.
    wbs = []
    cwbs = []
    for gp in range(2):
        wsl = weight[CO * gp : CO * gp + CO]  # [128, 32, 3, 3]
        blk = np.zeros((128, NPASS, CO), np.float32)
        for g in range(2):
            cosl = wsl[64 * g : 64 * g + 64]  # [64co, 32ci, 3, 3]
            for j in range(3):
                blk[32 * g : 32 * g + 32, j, 64 * g : 64 * g + 64] = cosl[:, :, 0, j].T
                blk[64 + 32 * g : 64 + 32 * g + 32, j, 64 * g : 64 * g + 64] = (
                    cosl[:, :, 1, j].T
                )
                blk[32 * g : 32 * g + 32, 3 + j, 64 * g : 64 * g + 64] = (
                    cosl[:, :, 2, j].T
                )
        wbs.append(blk)

        cwbv = np.empty((CDIM + 1, CO), np.float32)
        cwbv[:CDIM] = c_weight[CO * gp : CO * gp + CO].T
        cwbv[CDIM] = bias[CO * gp : CO * gp + CO]
        cwbs.append(cwbv)

    in_maps = []
    for core in range(N_CORES):
        gp, q = divmod(core, 4)
        cbv = np.empty((CDIM + 1, IMGS), np.float32)
        cbv[:CDIM] = c[IMGS * q : IMGS * q + IMGS].T
        cbv[CDIM] = 1.0
        in_maps.append(
            {
                "xs": np.ascontiguousarray(
                    xpad[IMGS * q : IMGS * q + IMGS, CI * gp : CI * gp + CI]
                ),
                "wb": wbs[gp],
                "cwb": cwbs[gp],
                "cb": cbv,
            }
        )
    return in_maps


def kernel(x, c, weight, bias, c_weight):
    x = np.asarray(x, np.float32)
    c = np.asarray(c, np.float32)
    weight = np.asarray(weight, np.float32)
    bias = np.asarray(bias, np.float32)
    c_weight = np.asarray(c_weight, np.float32)

    nc = _get_program()
    in_maps = _shard_inputs(x, c, weight, bias, c_weight)
    res = run_bass_kernel_spmd(nc, in_maps, list(range(N_CORES)), trace=False)

    out = np.empty((N, COUT, H, W), np.float32)
    for core in range(N_CORES):
        gp, q = divmod(core, 4)
        out[IMGS * q : IMGS * q + IMGS, CO * gp : CO * gp + CO] = res.results[core]["y"]
    return out


# revision 10
# speedup vs baseline: 10.4689x; 10.4689x over previous
"""ContextualConv2d Trainium2 kernel.

Problem: grouped 3x3 conv (N=32, 128ci -> 256co, groups=4, 56x56, pad 1)
plus per-(batch,channel) context bias: out = conv(x, w) + (c @ cwT)[n,co]
+ bias[co].

Sharding (8 cores): core = (group-pair gp in {0,1}) x (batch quarter q in
{0..3}). Each core computes 8 images x 128 out-channels; the 128 co of
groups {2gp, 2gp+1} depend only on ci [64gp, 64gp+64).

Matmul scheme (v3): the host ships x in bf16 with partitions 0..63
holding the image's 64 ci and partitions 64..127 the same data shifted
down one image row (pre-duplicated host-side; bf16 keeps HBM x traffic
at fp32-single-copy parity). A K=128 bf16 matmul then contracts TWO
kernel rows at once: lhsT rows 0..63 carry the (kh=0, kw)
block-diagonal weight block, rows 64..127 the (kh=1, kw) block. 3 such
passes (kw = 0,1,2) cover kh in {0,1}; 3 K=64 passes cover kh=2. 6
passes/tile instead of the 9 of the one-position-per-pass layout: PE
rows drop from 225792 to 150528 per core. Context/bias epilogue fused
into the PSUM->SBUF copy, alternating DVE/ACT engines.
"""

import numpy as np
from ml_dtypes import bfloat16 as np_bf16

from concourse import bass, mybir, tile
from concourse.vector_clock import ScopedClock
from concourse.bass_utils import run_bass_kernel_spmd

N, CIN, H, W = 32, 128, 56, 56
COUT, KH, KW = 256, 3, 3
GROUPS = 4
CDIM = 64
HP, WP = H + 2, W + 2
ROWS = 8              # output rows per n-tile
NT = H // ROWS        # 7 n-tiles per image
NFREE = ROWS * W      # 448 <= 512 fp32 PSUM bank limit
N_CORES = 8
IMGS = N // 4         # 8 images per core
CI = CIN // 2         # 64 input channels per core (2 groups)
CO = COUT // 2        # 128 output channels per core (2 groups)
NPASS = 6             # 3 dual-row (K=128) + 3 single-row (K=64) passes


class _TC(tile.TileContext):
    """This container's walrus accepts only one sem wait on a Drain
    (CTRL) instruction; TileContext's tail drain aggregates one wait per
    outstanding semaphore. Split them across sequential drains."""

    def _drain_and_barrier(self, tick_clock, wait_clock):
        drain_inst = self.nc.sync.drain()
        wait_clock.add_sem_waits(
            drain_inst.ins, ScopedClock({None: tick_clock.global_clock})
        )
        si = drain_inst.ins.sync_info
        if si is not None and len(si.on_wait) > 1:
            waits = list(si.on_wait)
            si.on_wait.clear()
            si.on_wait.append(waits[0])
            for w in waits[1:]:
                d2 = self.nc.sync.drain()
                d2.ins.sync_info = mybir.SyncInfo(on_wait=[w], on_update=[])
        self.nc.all_engine_barrier()
        assert self.sems is not None
        popped = self.nc._tile_sem_poison_stack.pop()
        assert popped is self._sem_poison
        self.nc.clear_and_free_semaphores(list(self.sems.allocated().values()))
        self.nc.all_engine_barrier()


_ws_ctr = [0]


def _split_waits(nc):
    """Walrus here caps sem waits at one per instruction; hoist extras
    onto injected same-engine NoOps placed just before the owner."""
    for fn in nc.m.functions:
        for blk in fn.blocks:
            insts = blk.instructions
            out = []
            changed = False
            for inst in insts:
                si = getattr(inst, "sync_info", None)
                if si is not None and si.on_wait and len(si.on_wait) > 1:
                    waits = list(si.on_wait)
                    for w in waits[:-1]:
                        _ws_ctr[0] += 1
                        out.append(
                            mybir.InstNoOp(
                                name=f"WSNOP-{_ws_ctr[0]}",
                                engine=inst.engine,
                                ins=[],
                                outs=[],
                                sync_info=mybir.SyncInfo(on_wait=[w], on_update=[]),
                                debug=inst.debug,
                            )
                        )
                        changed = True
                    si.on_wait.clear()
                    si.on_wait.append(waits[-1])
                out.append(inst)
            if changed:
                insts.clear()
                insts.extend(out)
    return nc


def build_program(loop_n: int = 0):
    """loop_n > 0 builds a benchmark variant: the conv body repeats
    loop_n times inside a hardware For_i so device time dominates the
    (RPC/transfer-heavy) wall clock. loop_n=0 is the production kernel."""
    f32 = mybir.dt.float32
    f32r = mybir.dt.float32r
    bf16 = mybir.dt.bfloat16
    nc = bass.Bass("TRN2", target_bir_lowering=False, debug=False)
    xs = nc.declare_dram_parameter("xs", [IMGS, 128, HP, WP], bf16, isOutput=False)
    wb = nc.declare_dram_parameter("wb", [128, NPASS, CO], bf16, isOutput=False)
    cwb = nc.declare_dram_parameter("cwb", [CDIM + 1, CO], f32r, isOutput=False)
    cb = nc.declare_dram_parameter("cb", [CDIM + 1, IMGS], f32r, isOutput=False)
    y = nc.declare_dram_parameter("y", [IMGS, CO, H, W], f32, isOutput=True)

    with _TC(nc) as tc:
        with (
            tc.tile_pool(name="wp", bufs=1) as wpool,
            tc.tile_pool(name="xp", bufs=4) as xpool,
            tc.tile_pool(name="op", bufs=3) as opool,
            tc.tile_pool(name="psp", bufs=6, space="PSUM") as pspool,
            tc.tile_pool(name="psc", bufs=1, space="PSUM") as pscpool,
        ):
            wt = wpool.tile([128, NPASS, CO], bf16)
            nc.sync.dma_start(wt[:], wb[:])
            cwbt = wpool.tile([CDIM + 1, CO], f32r)
            nc.sync.dma_start(cwbt[:], cwb[:])
            cbt = wpool.tile([CDIM + 1, IMGS], f32r)
            nc.sync.dma_start(cbt[:], cb[:])

            # bctx[co, n] = sum_d c_weight[co,d] c[n,d] + bias[co]
            psc = pscpool.tile([CO, IMGS], f32)
            nc.tensor.matmul(psc[:, :], cwbt[:], cbt[:], start=True, stop=True)
            bctx = wpool.tile([CO, IMGS], f32)
            nc.vector.tensor_copy(bctx[:], psc[:, :])

            def conv_body():
                for i in range(IMGS):
                    xt = xpool.tile([128, HP, WP], bf16, name=f"xt{i}", tag="xt")
                    nc.sync.dma_start(xt[:], xs[i])
                    ot = opool.tile([128, H * W], f32, name=f"ot{i}", tag="ot")
                    for t in range(NT):
                        ps = pspool.tile([128, NFREE], f32, name=f"ps{i}_{t}", tag="ps")
                        h0 = t * ROWS
                        for j in range(3):
                            # lower: (kh=0, kw=j); upper: (kh=1, kw=j)
                            nc.tensor.matmul(
                                ps[:, :],
                                wt[:, j, :],
                                xt[:, h0 : h0 + ROWS, j : j + W],
                                start=(j == 0),
                                stop=False,
                            )
                        for j in range(3):
                            # (kh=2, kw=j), lower 64 partitions only
                            nc.tensor.matmul(
                                ps[:, :],
                                wt[0:64, 3 + j, :],
                                xt[0:64, h0 + 2 : h0 + 2 + ROWS, j : j + W],
                                start=False,
                                stop=(j == 2),
                            )
                        o = ot[:, t * NFREE : (t + 1) * NFREE]
                        if t % 2 == 0:
                            nc.vector.tensor_scalar_add(o, ps[:, :], bctx[:, i : i + 1])
                        else:
                            nc.scalar.activation(
                                o, ps[:, :], mybir.ActivationFunctionType.Identity,
                                bias=bctx[:, i : i + 1],
                            )
                    nc.sync.dma_start(y[i].rearrange("c h w -> c (h w)"), ot[:])

            if loop_n > 0:
                with tc.For_i(0, loop_n, 1, hint_engines=(mybir.EngineType.PE,)):
                    conv_body()
            else:
                conv_body()
    _split_waits(nc)
    return nc


_prog_cache = {}


def _get_program():
    if "nc" not in _prog_cache:
        _prog_cache["nc"] = build_program()
    return _prog_cache["nc"]


def _shard_inputs(x, c, weight, bias, c_weight):
    """Build the per-core input dicts (pure layout prep, no math)."""
    # bf16 padded x, duplicated: channel rows 64..127 hold the same 64
    # channels shifted down one image row (xdup[64+ci, h] = xpad[ci, h+1]).
    xpad = np.zeros((N, CIN, HP, WP), np_bf16)
    xpad[:, :, 1 : H + 1, 1 : W + 1] = x.astype(np_bf16)
    xdup = np.zeros((N, 2, CIN, HP, WP), np_bf16)
    xdup[:, 0] = xpad
    xdup[:, 1, :, 0 : HP - 1] = xpad[:, :, 1:HP]

    # Pass-major block-diagonal weights for each group pair.
    # Pass j in 0..2: rows 0..63 = (kh=0, kw=j) block, rows 64..127 =
    # (kh=1, kw=j) block. Pass 3+j: rows 0..63 = (kh=2, kw=j), rows
    # 64..127 unused (K=64 matmul). Block: ci_loc 32g..32g+31 ->
    # co_loc 64g..64g+63 for g in {0,1}.
    wbs = []
    cwbs = []
    for gp in range(2):
        wsl = weight[CO * gp : CO * gp + CO].astype(np_bf16)  # [128, 32, 3, 3]
        blk = np.zeros((128, NPASS, CO), np_bf16)
        for g in range(2):
            cosl = wsl[64 * g : 64 * g + 64]  # [64co, 32ci, 3, 3]
            for j in range(3):
                blk[32 * g : 32 * g + 32, j, 64 * g : 64 * g + 64] = cosl[:, :, 0, j].T
                blk[64 + 32 * g : 64 + 32 * g + 32, j, 64 * g : 64 * g + 64] = (
                    cosl[:, :, 1, j].T
                )
                blk[32 * g : 32 * g + 32, 3 + j, 64 * g : 64 * g + 64] = (
                    cosl[:, :, 2, j].T
                )
        wbs.append(blk)

        cwbv = np.empty((CDIM + 1, CO), np.float32)
        cwbv[:CDIM] = c_weight[CO * gp : CO * gp + CO].T
        cwbv[CDIM] = bias[CO * gp : CO * gp + CO]
        cwbs.append(cwbv)

    in_maps = []
    for core in range(N_CORES):
        gp, q = divmod(core, 4)
        cbv = np.empty((CDIM + 1, IMGS), np.float32)
        cbv[:CDIM] = c[IMGS * q : IMGS * q + IMGS].T
        cbv[CDIM] = 1.0
        in_maps.append(
            {
                "xs": np.ascontiguousarray(
                    xdup[IMGS * q : IMGS * q + IMGS, :, CI * gp : CI * gp + CI]
                ).reshape(IMGS, 128, HP, WP),
                "wb": wbs[gp],
                "cwb": cwbs[gp],
                "cb": cbv,
            }
        )
    return in_maps


def kernel(x, c, weight, bias, c_weight):
    x = np.asarray(x, np.float32)
    c = np.asarray(c, np.float32)
    weight = np.asarray(weight, np.float32)
    bias = np.asarray(bias, np.float32)
    c_weight = np.asarray(c_weight, np.float32)

    nc = _get_program()
    in_maps = _shard_inputs(x, c, weight, bias, c_weight)
    res = run_bass_kernel_spmd(nc, in_maps, list(range(N_CORES)), trace=False)

    out = np.empty((N, COUT, H, W), np.float32)
    for core in range(N_CORES):
        gp, q = divmod(core, 4)
        out[IMGS * q : IMGS * q + IMGS, CO * gp : CO * gp + CO] = res.results[core]["y"]
    return out
